# revision 2
# baseline (speedup 1.0000x reference)
"""ColBERT MaxSim contrastive loss on 8 Trainium2 NeuronCores.

Sharding: doc-parallel (each core scores ALL 64*32 query tokens against its
8-doc shard = 8192 doc tokens). Per core the work is 128 (m-tile, doc) units
(16 m-tiles of 128 query rows x 8 docs), processed in 43 groups of <=3 units:

  - PE: per unit, 2 bf16 matmuls qT[128,128].T @ dT[128,512] -> the unit's
    A-half (tokens 0-511, kept in PSUM) and B-half (tokens 512-1023, staged
    in 2 rotating PSUM banks).
  - ScalarE: drains B-halves PSUM->SBUF (one 1024-elem + one 512-elem copy
    per group).
  - VectorE: ONE custom paged op per group: TT_MAXMAX_PG_ANT streams
    (A_psum[i], B_sbuf[i]) pairs at 1 pair/cycle, keeps a running
    max(pair...) scan that RESETS at each 512-element page (SUB_DIM_DONE
    step state) and writes only page-last values (out_last_subdim_enable)
    -> per-(row, doc) maxes, 3 per op, no accumulator-readout companion.

PSUM budget: 2x[128,3,512] A tiles (6 banks) + 1x[128,2,512] B staging
(2 banks) = 8 banks exactly.

Host gathers the 8 per-core dmax[128 rows, 128 units] tiles, does the tiny
n-sum over 32 query tokens, length-normalize, and the cross-entropy.
"""

import numpy as np

B, NTOK, DIM = 64, 32, 128
C, S = 64, 1024
NCORES = 8
CSHARD = C // NCORES              # 8 docs per core
ROWS = B * NTOK                   # 2048 score rows
MTILES = ROWS // 128              # 16
DCOLS = CSHARD * S                # 8192 doc-token columns per core
UNITS = MTILES * CSHARD           # 128 (m-tile, doc) units
GROUP = 3                         # units per DVE op (3 PSUM banks of A-halves)
TEMPERATURE = 0.02

_CACHE = {}


def _register_ttmax_paged():
    """Custom DVE op: out[p, s] = max_n max(in0[p, s, n], in1[p, s*N + n]).

    Built from Spec(body=Scan(MAX, maxx(Src0, Src1), init=MaxNeg)); the
    lowered [seed, steady] FSM is hand-extended with a SUB_DIM_DONE step
    state that resets the scan flop to the boundary element's pair-max, and
    out_last_subdim_enable so only page-last scan values are written (one
    output per page). Validated bit-exact on hardware for PSUM and SBUF in0.
    """
    import copy

    from concourse import dve_ops as DO
    from concourse.dve_spec import AluOp, MaxNeg, Scan, Spec, Src0, Src1, lower, maxx
    from concourse.dve_uop import AluInp, DveOpSpec, ENABLE, Trigger

    NAME = "TT_MAXMAX_PG_ANT"
    for o in DO.OPS:
        if o.name == NAME:
            return o

    def _ref(in0, in1, c0, c1, c2):
        P = in0.shape[0]
        N = in0.shape[-1]
        Spg = int(np.prod(in0.shape[1:-1]))
        a = np.asarray(in0, np.float32).reshape(P, Spg, N)
        b = np.asarray(in1, np.float32).reshape(P, Spg, N)
        return np.maximum(a, b).max(axis=-1)

    spec = Spec(body=Scan(AluOp.MAX, maxx(Src0, Src1), init=MaxNeg), reference=_ref)
    uops = lower(spec, ver="v3")
    assert len(uops) == 2
    seed, steady = uops
    steady.trigger = (Trigger.SRC_TENSOR_DONE, Trigger.SUB_DIM_DONE, Trigger.NONE)
    steady.next_uop = (0, 2, 0)
    steady.out_last_subdim_enable = ENABLE
    step = copy.deepcopy(steady)
    step.trigger = (Trigger.SRC_TENSOR_DONE, Trigger.SUB_DIM_DONE, Trigger.COUNT)
    step.next_uop = (0, 2, 1)
    step.repeat_count = 1
    dp = step.datapath_config[1]
    dp.op = AluOp.BYPASS
    dp.alu_src0 = AluInp.PREV_ALU_OUT
    dp.alu_src1 = AluInp.PREV_ALU_OUT

    op = DO.DveOp(NAME, spec, subdim=True, uops_sha={})
    DO.OPS.append(op)
    DO.CUSTOM_DVE_SPECS[op.name] = op.spec
    DO._SUB_OPCODE_FOR_NAME[op.name] = DO._CUSTOM_DVE_ROW_BASE + len(DO.OPS) - 1
    ds = DveOpSpec(
        name=NAME,
        opcode=DO.get_dve_sub_opcode(NAME),
        uops=[seed, steady, step],
        rd1_en=True,
    )
    ds.validate("v3")
    op.uops_sha["v3"] = ds.sha("v3")
    DO._COMPILE_CACHE[(NAME, "v3")] = ds
    return op


def _build_nc():
    import concourse.bacc as bacc
    import concourse.tile as tile
    from concourse import mybir

    f32 = mybir.dt.float32
    bf16 = mybir.dt.bfloat16
    op = _register_ttmax_paged()

    nc = bacc.Bacc("TRN2", target_bir_lowering=False, debug=False)
    qT_d = nc.dram_tensor("qT", [DIM, ROWS], bf16, kind="ExternalInput").ap()
    dT_d = nc.dram_tensor("dT", [DIM, DCOLS], bf16, kind="ExternalInput").ap()
    dmax_d = nc.dram_tensor("dmax", [128, UNITS], f32, kind="ExternalOutput").ap()

    with tile.TileContext(nc) as tc:
        with (
            tc.tile_pool(name="const", bufs=1) as cpool,
            tc.tile_pool(name="b1", bufs=3) as b1_pool,
            tc.tile_pool(name="pa", bufs=2, space="PSUM") as pa_pool,
            tc.tile_pool(name="pb", bufs=1, space="PSUM") as pb_pool,
        ):
            qT_sb = cpool.tile([DIM, ROWS], bf16)
            dT_sb = cpool.tile([DIM, DCOLS], bf16)
            wsb = cpool.tile([128, 512], bf16)
            dmax_sb = cpool.tile([128, UNITS], f32)
            nc.gpsimd.memset(wsb[:], 0.0)

            # DMA staging: just what group 0 needs first, then the rest in
            # big chunks (each dma_start issue costs ~0.6us on the queue).
            nc.sync.dma_start(qT_sb[:, 0:128], qT_d[:, 0:128])
            nc.sync.dma_start(dT_sb[:, 0:1024], dT_d[:, 0:1024])
            nc.sync.dma_start(dT_sb[:, 1024:2048], dT_d[:, 1024:2048])
            nc.sync.dma_start(dT_sb[:, 2048:3072], dT_d[:, 2048:3072])
            nc.gpsimd.dma_start(qT_sb[:, 128:2048], qT_d[:, 128:2048])
            nc.sync.dma_start(dT_sb[:, 3072:4096], dT_d[:, 3072:4096])
            nc.sync.dma_start(dT_sb[:, 4096:6144], dT_d[:, 4096:6144])
            nc.sync.dma_start(dT_sb[:, 6144:8192], dT_d[:, 6144:8192])

            # B-half staging: one [128, 2, 512] tile = 2 PSUM banks, also the
            # HAM warm-up target while the input DMAs run.
            Bt = pb_pool.tile([128, 2, 512], f32)
            for _ in range(12):
                nc.tensor.matmul(Bt[:, 0, :], wsb[:, 0:128], wsb[:], start=True, stop=True)

            for g in range((UNITS + GROUP - 1) // GROUP):
                units = list(range(g * GROUP, min((g + 1) * GROUP, UNITS)))
                nu = len(units)
                A = pa_pool.tile([128, GROUP, 512], f32, tag="A")
                in1 = b1_pool.tile([128, GROUP * 512], f32, tag="b1")

                # B-halves first so ScalarE can drain while the A matmuls run
                for j, u in enumerate(units[:2]):
                    m, d = u // CSHARD, u % CSHARD
                    nc.tensor.matmul(
                        Bt[:, j, :],
                        qT_sb[:, m * 128:(m + 1) * 128],
                        dT_sb[:, d * 1024 + 512:d * 1024 + 1024],
                        start=True,
                        stop=True,
                    )
                nc.scalar.copy(in1[:, 0:1024], Bt[:, :, :])
                for j, u in enumerate(units):
                    m, d = u // CSHARD, u % CSHARD
                    nc.tensor.matmul(
                        A[:, j, :],
                        qT_sb[:, m * 128:(m + 1) * 128],
                        dT_sb[:, d * 1024:d * 1024 + 512],
                        start=True,
                        stop=True,
                    )
                if nu == 3:
                    m, d = units[2] // CSHARD, units[2] % CSHARD
                    nc.tensor.matmul(
                        Bt[:, 0, :],
                        qT_sb[:, m * 128:(m + 1) * 128],
                        dT_sb[:, d * 1024 + 512:d * 1024 + 1024],
                        start=True,
                        stop=True,
                    )
                    nc.scalar.copy(in1[:, 1024:1536], Bt[:, 0, :])

                nc.vector._custom_dve(
                    op,
                    out=dmax_sb[:, units[0]:units[0] + nu],
                    in0=A[:, 0:nu, :],
                    in1=in1[:, 0:nu * 512],
                )

            nc.sync.dma_start(dmax_d[:, :], dmax_sb[:])

    nc.compile()
    return nc


def _host_inputs(q, d):
    import ml_dtypes

    bf = ml_dtypes.bfloat16
    qT = np.ascontiguousarray(q.transpose(2, 0, 1).reshape(DIM, ROWS)).astype(bf)
    in_maps = []
    for k in range(NCORES):
        dTk = np.ascontiguousarray(
            d[k * CSHARD:(k + 1) * CSHARD].transpose(2, 0, 1).reshape(DIM, DCOLS)
        ).astype(bf)
        in_maps.append({"qT": qT, "dT": dTk})
    return in_maps


def _finish_host(dmaxes, q, offset):
    # dmax[k]: [128 rows, 128 units], unit = m*8 + d; global row = m*128 + r
    # -> per-core scores [64, 8] = sum over the 32 query tokens of each b
    per_core = []
    for k in range(NCORES):
        m_r_d = dmaxes[k].astype(np.float64).reshape(128, MTILES, CSHARD)
        rows_d = m_r_d.transpose(1, 0, 2).reshape(ROWS, CSHARD)
        per_core.append(rows_d.reshape(B, NTOK, CSHARD).sum(axis=1))
    S_mat = np.concatenate(per_core, axis=1)  # [64, 64]
    lengths = (q[:, :, 0] != 0).sum(axis=1).astype(np.float64)
    S_mat = S_mat / lengths[:, None]
    logits = S_mat / TEMPERATURE
    m = logits.max(axis=1, keepdims=True)
    logp = logits - m - np.log(np.exp(logits - m).sum(axis=1, keepdims=True))
    labels = np.arange(B) + offset
    return np.float32(-np.mean(logp[np.arange(B), labels]))


def kernel(**inputs):
    from concourse import bass_utils

    q = np.ascontiguousarray(np.asarray(inputs["query_embeddings"], dtype=np.float32))
    d = np.ascontiguousarray(np.asarray(inputs["doc_embeddings"], dtype=np.float32))
    offset = int(np.asarray(inputs["offset"]))
    assert q.shape == (B, NTOK, DIM) and d.shape == (C, S, DIM)

    if "nc" not in _CACHE:
        _CACHE["nc"] = _build_nc()
    nc = _CACHE["nc"]

    in_maps = _host_inputs(q, d)
    res = bass_utils.run_bass_kernel_spmd(nc, in_maps, core_ids=list(range(NCORES)))
    dmaxes = [res.results[k]["dmax"] for k in range(NCORES)]
    return _finish_host(dmaxes, q, offset)
